# revision 3
# baseline (speedup 1.0000x reference)
"""Multi-head attention (B=4, S=1024, D=1024, H=16) on 8 trn2 NeuronCores.

Sharding: core c handles batch b = c//2, query rows [sh*512, (sh+1)*512) with
sh = c%2. Each core computes all 16 heads for its 512 query rows against the
full 1024 keys/values of its batch, plus the output projection for its rows.
No collectives needed; host concatenates the per-core output shards.

Layout strategy per core:
  - q/k/v slices are PE-transposed into [D_model-on-partition, seq] form
    (fp32 has no DMA-transpose path on trn2).
  - QT = (Wq^T x^T)  -> [head_cols 128/tile, 512 q]   (transposed layout)
  - KT = (Wk^T x^T)  -> [head_cols, 1024 k]           (transposed layout)
  - V  natural       -> [1024 k, head_cols], stored with a constant-1 column
    appended per head ([k, 16*65]) so the PV matmul also produces the softmax
    denominator (sum_k exp) in its last output column.
  - scores are computed in BOTH orientations on the PE (cheaper than
    transposing the 4 MiB exp(scores) matrix):
      natural  [q, k]: softmax row-sums come free via ScalarE accum_out;
                       P = exp * 1/s written straight to DRAM.
      transposed [k, q]: feeds the PV matmul as the stationary operand.
    Softmax skips the max-subtraction (logits are ~N(0,1) by construction;
    exp is computed in fp32 and normalization cancels any constant shift).
  - ctx [q, 64] is normalized with the denominator from its own matmul
    (self-consistent), PE-transposed into ctxT [128 dconcat, q] tiles, and
    fed to the Wo matmul producing natural-layout output rows.
"""

import numpy as np

import concourse.bass as bass
import concourse.mybir as mybir
import concourse.tile as tile
from concourse import bacc
from concourse.bass_utils import run_bass_kernel_spmd
from concourse.masks import make_identity

F32 = mybir.dt.float32
AF = mybir.ActivationFunctionType

B, S, D, H, DK = 4, 1024, 1024, 16, 64
N_CORES = 8
SH = S // 2          # query rows per core
P = 128              # partitions
KD = D // P          # 8 tiles along the model/contraction dim
QT_T = SH // P       # 4 query tiles per core
KT_T = S // P        # 8 key tiles
SCALE = 1.0 / np.sqrt(DK)

# matmul dtypes per stage (perf knobs; float32 = exact, float32r = fast TF32-ish)
PROJ_DT = F32
SCORE_DT = F32
CTX_DT = F32
WO_DT = F32


def _bias_cols(pool, nc, dram_vec, name):
    """Load a [D] bias as [128, KD] (partition-major) for per-partition adds."""
    t = pool.tile([P, KD], F32, tag=name)
    nc.sync.dma_start(t[:], dram_vec.rearrange("(c p) -> p c", p=P))
    return t


def build_kernel():
    nc = bacc.Bacc(None, target_bir_lowering=False)

    xq = nc.dram_tensor("xq", [SH, D], F32, kind="ExternalInput")
    xk = nc.dram_tensor("xk", [S, D], F32, kind="ExternalInput")
    xv = nc.dram_tensor("xv", [S, D], F32, kind="ExternalInput")
    wq = nc.dram_tensor("wq", [D, D], F32, kind="ExternalInput")
    wk = nc.dram_tensor("wk", [D, D], F32, kind="ExternalInput")
    wv = nc.dram_tensor("wv", [D, D], F32, kind="ExternalInput")
    wo = nc.dram_tensor("wo", [D, D], F32, kind="ExternalInput")
    bq = nc.dram_tensor("bq", [D], F32, kind="ExternalInput")
    bk = nc.dram_tensor("bk", [D], F32, kind="ExternalInput")
    bv = nc.dram_tensor("bv", [D], F32, kind="ExternalInput")
    bo = nc.dram_tensor("bo", [D], F32, kind="ExternalInput")
    attn_o = nc.dram_tensor("attn", [H, SH, S], F32, kind="ExternalOutput")
    out_o = nc.dram_tensor("out", [SH, D], F32, kind="ExternalOutput")

    with tile.TileContext(nc) as tc:
        import contextlib

        with contextlib.ExitStack() as ctx:
            singles = ctx.enter_context(tc.tile_pool(name="singles", bufs=1))
            qt_pool = ctx.enter_context(tc.tile_pool(name="qtp", bufs=KD))
            kt_pool = ctx.enter_context(tc.tile_pool(name="ktp", bufs=KD))
            vn_pool = ctx.enter_context(tc.tile_pool(name="vnp", bufs=KD))
            ct_pool = ctx.enter_context(tc.tile_pool(name="ctp", bufs=KD))
            w_pool = ctx.enter_context(tc.tile_pool(name="wp", bufs=KD))
            small = ctx.enter_context(tc.tile_pool(name="small", bufs=24))
            ps512 = ctx.enter_context(tc.tile_pool(name="ps512", bufs=4, space="PSUM"))
            psctx = ctx.enter_context(tc.tile_pool(name="psctx", bufs=2, space="PSUM"))
            pstr = ctx.enter_context(tc.tile_pool(name="pstr", bufs=2, space="PSUM"))

            ident = singles.tile([P, P], F32, tag="ident")
            make_identity(nc, ident)
            ones_row = singles.tile([1, P], F32, tag="ones_row")
            nc.vector.memset(ones_row, 1.0)
            bqs = _bias_cols(singles, nc, bq, "bqs")
            bks = _bias_cols(singles, nc, bk, "bks")
            bv_row = singles.tile([1, D], F32, tag="bv_row")
            nc.sync.dma_start(bv_row[:], bv.rearrange("(o d) -> o d", o=1))
            bo_row = singles.tile([1, D], F32, tag="bo_row")
            nc.sync.dma_start(bo_row[:], bo.rearrange("(o d) -> o d", o=1))

            # persistent per-core tensors
            QT = [qt_pool.tile([P, SH], F32, tag="qt", name=f"QT{i}") for i in range(KD)]
            KT = [kt_pool.tile([P, S], F32, tag="kt", name=f"KT{i}") for i in range(KD)]
            VN = [vn_pool.tile([P, H * (DK + 1)], F32, tag="vn", name=f"VN{i}") for i in range(KD)]
            CT = [ct_pool.tile([P, SH], F32, tag="ct", name=f"CT{i}") for i in range(KD)]

            # constant-1 column per head in VN
            for ri in range(KD):
                v3 = VN[ri].rearrange("p (h x) -> p h x", x=DK + 1)
                nc.vector.memset(v3[:, :, DK : DK + 1], 1.0)

            evac_i = 0

            def evac_copy(dst, src):
                nonlocal evac_i
                if evac_i % 2 == 0:
                    nc.vector.tensor_copy(dst, src)
                else:
                    nc.scalar.activation(dst, src, AF.Copy)
                evac_i += 1

            def mm(ps, lhsT, rhs, dt, start, stop):
                if dt is not F32:
                    lhsT = lhsT.bitcast(dt)
                    rhs = rhs.bitcast(dt)
                nc.tensor.matmul(ps, lhsT, rhs, start=start, stop=stop)

            def load_transposed(x_dram, nrows, xt_tiles, xl_pool):
                """xt_tiles[ci][:, r*128:(r+1)*128] = x[r*128:(r+1)*128, ci*128:...].T"""
                for ri in range(nrows // P):
                    xl = xl_pool.tile([P, D], F32, tag="xl")
                    nc.sync.dma_start(xl[:], x_dram[ri * P : (ri + 1) * P, :])
                    for ci in range(KD):
                        pt = pstr.tile([P, P], F32, tag="pstr")
                        nc.tensor.transpose(pt[:], xl[:, ci * P : (ci + 1) * P], ident[:])
                        evac_copy(xt_tiles[ci][:, ri * P : (ri + 1) * P], pt[:])

            def load_w(w_dram):
                tiles = [w_pool.tile([P, D], F32, tag="w", name=f"w{ki}") for ki in range(KD)]
                for ki in range(KD):
                    nc.sync.dma_start(tiles[ki][:], w_dram[ki * P : (ki + 1) * P, :])
                return tiles

            # ---------------- phase A+B: transposes + projections ----------------
            with tc.tile_pool(name="xt", bufs=KD) as xt_pool, \
                 tc.tile_pool(name="xl", bufs=3) as xl_pool:
                # Q
                wq_t = load_w(wq)
                qt_x = [xt_pool.tile([P, SH], F32, tag="xt", name=f"qtx{i}") for i in range(KD)]
                load_transposed(xq, SH, qt_x, xl_pool)
                for ci in range(KD):
                    ps = ps512.tile([P, SH], F32, tag="ps")
                    for ki in range(KD):
                        mm(ps[:], wq_t[ki][:, ci * P : (ci + 1) * P], qt_x[ki][:],
                           PROJ_DT, start=(ki == 0), stop=(ki == KD - 1))
                    nc.vector.tensor_scalar_add(QT[ci][:], ps[:], bqs[:, ci : ci + 1])
                # K
                wk_t = load_w(wk)
                kt_x = [xt_pool.tile([P, S], F32, tag="xt", name=f"ktx{i}") for i in range(KD)]
                load_transposed(xk, S, kt_x, xl_pool)
                for ci in range(KD):
                    for ni in range(2):
                        ps = ps512.tile([P, SH], F32, tag="ps")
                        for ki in range(KD):
                            mm(ps[:], wk_t[ki][:, ci * P : (ci + 1) * P],
                               kt_x[ki][:, ni * SH : (ni + 1) * SH],
                               PROJ_DT, start=(ki == 0), stop=(ki == KD - 1))
                        nc.vector.tensor_scalar_add(
                            KT[ci][:, ni * SH : (ni + 1) * SH], ps[:], bks[:, ci : ci + 1]
                        )
                # V (natural layout, interleaved with ones columns)
                wv_t = load_w(wv)
                vt_x = [xt_pool.tile([P, S], F32, tag="xt", name=f"vtx{i}") for i in range(KD)]
                load_transposed(xv, S, vt_x, xl_pool)
                for ri in range(KD):
                    for ni in range(2):
                        ps = ps512.tile([P, SH], F32, tag="ps")
                        for ki in range(KD):
                            mm(ps[:], vt_x[ki][:, ri * P : (ri + 1) * P],
                               wv_t[ki][:, ni * SH : (ni + 1) * SH],
                               PROJ_DT, start=(ki == 0), stop=False)
                        mm(ps[:], ones_row[:], bv_row[:, ni * SH : (ni + 1) * SH],
                           PROJ_DT, start=False, stop=True)
                        dst = VN[ri].rearrange("p (h x) -> p h x", x=DK + 1)
                        evac_copy(
                            dst[:, ni * 8 : (ni + 1) * 8, 0:DK],
                            ps[:].rearrange("p (h d) -> p h d", d=DK),
                        )

            # ---------------- phase C: attention ----------------
            wo_t = load_w(wo)  # reuses w pool slots as Wv readers finish
            with tc.tile_pool(name="ep", bufs=10) as e_pool, \
                 tc.tile_pool(name="en", bufs=3) as en_pool:
                for h in range(H):
                    hc, hp = h // 2, (h % 2) * DK
                    qts = QT[hc][hp : hp + DK, :]      # [64, 512]
                    kts = KT[hc][hp : hp + DK, :]      # [64, 1024]

                    # transposed scores -> E' tiles [128 k, 512 q]
                    Ep = []
                    for ri in range(KT_T):
                        ps = ps512.tile([P, SH], F32, tag="ps")
                        mm(ps[:], kts[:, ri * P : (ri + 1) * P], qts,
                           SCORE_DT, start=True, stop=True)
                        et = e_pool.tile([P, SH], F32, tag="ep")
                        nc.scalar.activation(et[:], ps[:], AF.Exp, scale=SCALE)
                        Ep.append(et)

                    # ctx (+denominator) per query tile, then ctxT
                    for qi in range(QT_T):
                        pc = psctx.tile([P, DK + 1], F32, tag="pc")
                        for ri in range(KT_T):
                            mm(pc[:], Ep[ri][:, qi * P : (qi + 1) * P],
                               VN[ri][:, h * (DK + 1) : (h + 1) * (DK + 1)],
                               CTX_DT, start=(ri == 0), stop=(ri == KT_T - 1))
                        r = small.tile([P, 1], F32, tag="r")
                        nc.vector.reciprocal(r[:], pc[:, DK : DK + 1])
                        cn = small.tile([P, DK], F32, tag="cn")
                        nc.vector.tensor_scalar_mul(cn[:], pc[:, 0:DK], r[:])
                        pt = pstr.tile([P, P], F32, tag="pstr")
                        nc.tensor.transpose(pt[0:DK, :], cn[:], ident[:])
                        evac_copy(CT[hc][hp : hp + DK, qi * P : (qi + 1) * P], pt[0:DK, :])

                    # natural scores -> P rows straight to DRAM
                    for qi in range(QT_T):
                        en = en_pool.tile([P, S], F32, tag="en")
                        ssum = small.tile([P, 1], F32, tag="ss")
                        for ni in range(2):
                            ps = ps512.tile([P, SH], F32, tag="ps")
                            mm(ps[:], qts[:, qi * P : (qi + 1) * P],
                               kts[:, ni * SH : (ni + 1) * SH],
                               SCORE_DT, start=True, stop=True)
                            acc = small.tile([P, 1], F32, tag=f"acc{ni}")
                            nc.scalar.activation(
                                en[:, ni * SH : (ni + 1) * SH], ps[:], AF.Exp,
                                scale=SCALE, accum_out=acc[:],
                            )
                            if ni == 0:
                                acc0 = acc
                        nc.vector.tensor_add(ssum[:], acc0[:], acc[:])
                        rn = small.tile([P, 1], F32, tag="rn")
                        nc.vector.reciprocal(rn[:], ssum[:])
                        nc.vector.tensor_scalar_mul(en[:], en[:], rn[:])
                        nc.sync.dma_start(
                            attn_o[h, qi * P : (qi + 1) * P, :], en[:]
                        )

            # ---------------- phase D: output projection ----------------
            with tc.tile_pool(name="ob", bufs=3) as ob_pool:
                for qi in range(QT_T):
                    ob = ob_pool.tile([P, D], F32, tag="ob")
                    for ni in range(2):
                        ps = ps512.tile([P, SH], F32, tag="ps")
                        for ki in range(KD):
                            mm(ps[:], CT[ki][:, qi * P : (qi + 1) * P],
                               wo_t[ki][:, ni * SH : (ni + 1) * SH],
                               WO_DT, start=(ki == 0), stop=False)
                        mm(ps[:], ones_row[:], bo_row[:, ni * SH : (ni + 1) * SH],
                           WO_DT, start=False, stop=True)
                        evac_copy(ob[:, ni * SH : (ni + 1) * SH], ps[:])
                    nc.sync.dma_start(out_o[qi * P : (qi + 1) * P, :], ob[:])

    nc.compile()
    return nc


_NC_CACHE = None


def _get_nc():
    global _NC_CACHE
    if _NC_CACHE is None:
        _NC_CACHE = build_kernel()
    return _NC_CACHE


def kernel(q, k, v, Wq, bq, Wk, bk, Wv, bv, Wo, bo):
    q = np.ascontiguousarray(np.asarray(q, np.float32))
    k = np.ascontiguousarray(np.asarray(k, np.float32))
    v = np.ascontiguousarray(np.asarray(v, np.float32))
    Wq = np.ascontiguousarray(np.asarray(Wq, np.float32))
    Wk = np.ascontiguousarray(np.asarray(Wk, np.float32))
    Wv = np.ascontiguousarray(np.asarray(Wv, np.float32))
    Wo = np.ascontiguousarray(np.asarray(Wo, np.float32))
    bq = np.ascontiguousarray(np.asarray(bq, np.float32))
    bk = np.ascontiguousarray(np.asarray(bk, np.float32))
    bv = np.ascontiguousarray(np.asarray(bv, np.float32))
    bo = np.ascontiguousarray(np.asarray(bo, np.float32))

    nc = _get_nc()
    in_maps = []
    for c in range(N_CORES):
        b, sh = c // 2, c % 2
        in_maps.append({
            "xq": np.ascontiguousarray(q[b, sh * SH : (sh + 1) * SH, :]),
            "xk": k[b], "xv": v[b],
            "wq": Wq, "wk": Wk, "wv": Wv, "wo": Wo,
            "bq": bq, "bk": bk, "bv": bv, "bo": bo,
        })
    res = run_bass_kernel_spmd(nc, in_maps, core_ids=list(range(N_CORES)))

    out = np.empty((B, S, D), np.float32)
    attn = np.empty((B, H, S, S), np.float32)
    for c in range(N_CORES):
        b, sh = c // 2, c % 2
        out[b, sh * SH : (sh + 1) * SH, :] = res.results[c]["out"]
        attn[b, :, sh * SH : (sh + 1) * SH, :] = res.results[c]["attn"]
    return out, attn
